# revision 1
# baseline (speedup 1.0000x reference)
"""Trainium2 Bass kernel for nn_L2Net (Jeffress coincidence-detector SNN).

Contract: kernel(**inputs) takes the FULL unsharded inputs (numpy) and
returns the FULL (T, N, 1) float32 output.

Strategy: pure data parallelism over the batch axis N=32 -> 4 samples on
each of 8 NeuronCores (every state in the model is per-sample, so there
are no collectives). Inside a core:
  partitions = C (=128 channels), free dims = (n_local, d / k / t).
  - Jeffress LIF: 3 fused ops/step; pre-reset membranes streamed to SBUF,
    spikes * kint recovered afterwards with one fused compare-multiply and
    a strided reduce (chunked so the vi chain can start early).
  - IF neurons: 2 fused ops/step (scalar_tensor_tensor integrate + reset).
  - SynapseFilters: one masked tensor_tensor_scan per filter (decay mask
    carries 0.0 at segment starts to reset the recurrence between samples).
  - Linear layers of sqrt_model + the sum over C: PE matmuls.
  - Output q2: cumulative-sum scan, DMAed out once.
"""
import os
import sys

import numpy as np

sys.path.insert(0, "/opt/trn_rl_repo")

T, N, C, D = 64, 32, 128, 64
NCORES = 8
NL = N // NCORES          # samples per core
TAU = np.float32(20.0)    # jeffress LIF tau
F32 = np.float32

_cache = {}


def _build_program():
    import concourse.bass as bass
    import concourse.bacc as bacc
    import concourse.mybir as mybir
    import concourse.tile as tile

    dt = mybir.dt.float32
    op = mybir.AluOpType
    AP = bass.AP

    nc = bacc.Bacc("TRN2", target_bir_lowering=False, debug=False,
                   num_devices=NCORES)

    # ---------------- DRAM I/O ----------------
    x0R_d = nc.dram_tensor("x0r", [C, NL, 128], dt, kind="ExternalInput")
    x1P_d = nc.dram_tensor("x1p", [C, NL, 128], dt, kind="ExternalInput")
    kint_d = nc.dram_tensor("kint", [C, D], dt, kind="ExternalInput")
    # packed per-channel weights: cols 0:10 w1, 10:20 b1, 20:30 w2, 30 b2,
    # col 31 ones (for the C-sum matmul)
    wpk_d = nc.dram_tensor("wpk", [C, 32], dt, kind="ExternalInput")
    # packed sqrt-model weights on 32 partitions:
    # col 0 sw2T, col 1 sb0, col 2 sb1, cols 3:35 sw1T (sw1T[k,j]=sw1[j,k])
    spk_d = nc.dram_tensor("spk", [32, 35], dt, kind="ExternalInput")
    # row tile: cols 0:32 sw0 (as [1,32]), col 32 sb2
    srow_d = nc.dram_tensor("srow", [1, 33], dt, kind="ExternalInput")
    out_d = nc.dram_tensor("out", [NL, T], dt, kind="ExternalOutput")

    NT = NL * T               # 256
    NKT = NL * 10 * T         # 2560
    CH = 8                    # t-chunks for the jeffress bulk
    CHT = T // CH             # 8 steps per chunk

    with tile.TileContext(nc) as tc:
        with (
            tc.tile_pool(name="pool", bufs=1) as pool,
            tc.tile_pool(name="psum", bufs=1, space="PSUM") as psum,
        ):
            xa = pool.tile([C, NL, 128], dt)
            xb = pool.tile([C, NL, 128], dt)
            kint = pool.tile([C, D], dt)
            wpk = pool.tile([C, 32], dt)
            spk = pool.tile([32, 35], dt)
            srow = pool.tile([1, 33], dt)
            mask1 = pool.tile([C, NT], dt)
            mask40 = pool.tile([C, NKT], dt)
            cmask = pool.tile([1, NT], dt)

            for tl, dr in ((xa, x0R_d), (xb, x1P_d), (kint, kint_d),
                           (wpk, wpk_d), (spk, spk_d), (srow, srow_d)):
                nc.sync.dma_start(tl[:], dr[:])

            # filter decay masks built on-device: 0.5 everywhere with 0.0 at
            # each t-segment start (resets the scan between samples);
            # cmask likewise with 1.0 for the output cumsum.
            nc.vector.memset(mask1[:], 0.5)
            nc.vector.memset(
                mask1[:].rearrange("c (n t) -> c n t", t=T)[:, :, 0:1], 0.0)
            nc.gpsimd.memset(mask40[:], 0.5)
            nc.gpsimd.memset(
                mask40[:].rearrange("c (s t) -> c s t", t=T)[:, :, 0:1], 0.0)
            nc.vector.memset(cmask[:], 1.0)
            nc.vector.memset(
                cmask[:].rearrange("p (n t) -> p n t", t=T)[:, :, 0:1], 0.0)

            # jeffress state: GPSIMD builds u_t = a_t + b_t (tensor_tensor
            # is Pool-legal), DVE runs the 2-op fused LIF chain.
            vj = pool.tile([C, NL, D], dt)
            vjs = pool.tile([C, T, NL, D], dt)   # pre-reset membrane stream
            zc = pool.tile([C, T, NL], dt)       # jeffress->vi inputs

            vi = pool.tile([C, NL], dt)
            vis = pool.tile([C, NL, T], dt)      # (n,t) pre-reset stream
            f1s = pool.tile([C, NT], dt)
            tmp1 = pool.tile([C, NKT], dt)       # (n,k,t) v1 inputs
            v1 = pool.tile([C, NL, 10], dt)
            v1s = pool.tile([C, NL, 10, T], dt)  # (n,k,t)
            f2s = pool.tile([C, NKT], dt)
            m2 = pool.tile([C, NKT], dt)         # (n,t,k)
            red2 = pool.tile([C, NL, T], dt)
            v2 = pool.tile([C, NL], dt)
            v2s = pool.tile([C, NL, T], dt)
            fss = pool.tile([C, NT], dt)

            vs = pool.tile([1, NL], dt)
            vss = pool.tile([1, NL, T], dt)
            q0 = pool.tile([32, NL], dt)
            q0s = pool.tile([32, NL, T], dt)
            g1s = pool.tile([32, NT], dt)
            tq0 = pool.tile([32, NT], dt)
            tq1 = pool.tile([32, NT], dt)
            tq2 = pool.tile([1, NT], dt)
            q1 = pool.tile([32, NL], dt)
            q1s = pool.tile([32, NL, T], dt)
            g2s = pool.tile([32, NT], dt)
            q2s = pool.tile([1, NT], dt)

            sums_ps = psum.tile([1, NT], dt)
            q0_ps = psum.tile([32, NT], dt)
            q1_ps = psum.tile([32, NT], dt)
            q2_ps = psum.tile([1, NT], dt)

            for tl in (vj, vi, v1, v2, vs, q0, q1):
                nc.vector.memset(tl[:], 0.0)

            dec = F32(1.0) - F32(1.0) / TAU     # 0.95

            # ---------------- phase 1: jeffress LIF ----------------
            # GPSIMD: u_t = a_t + b_t (delay-line windows; pre-reversed /
            # padded / prescaled by 1/tau on the host).
            # DVE: vn = (vj*0.95) + u_t ; vj = (vn<1)*vn   (2 fused ops)
            # bulk (chunked): GPSIMD turns the membrane stream into
            # kint-weighted spikes in place, DVE reduces over d -> zc.
            for ch in range(CH):
                t0 = ch * CHT
                for t in range(t0, t0 + CHT):
                    u_t = pool.tile([C, NL, D], dt, tag="u", bufs=4)
                    nc.gpsimd.tensor_tensor(
                        u_t[:], xa[:, :, 63 - t:127 - t],
                        xb[:, :, t:t + 64], op.add)
                    vn = vjs[:, t]
                    nc.vector.scalar_tensor_tensor(vn, vj[:], float(dec),
                                                   u_t[:], op.mult, op.add)
                    nc.vector.scalar_tensor_tensor(vj[:], vn, 1.0, vn,
                                                   op.is_lt, op.mult)
                blk = vjs[:, t0:t0 + CHT].rearrange("c a b d -> c (a b) d")
                kb = kint[:].unsqueeze(1).broadcast_to((C, CHT * NL, D))
                nc.gpsimd.tensor_scalar(blk, blk, 1.0, None, op.is_ge)
                nc.gpsimd.tensor_tensor(blk, blk, kb, op.mult)
                nc.vector.tensor_reduce(
                    zc[:, t0:t0 + CHT], blk.rearrange(
                        "c (a b) d -> c a b d", a=CHT),
                    mybir.AxisListType.X, op.add)

            # ---------------- phase 2: vi integrate-and-fire -------------
            for t in range(T):
                vn = vis[:, :, t]
                nc.vector.tensor_tensor(vn, vi[:], zc[:, t], op.add)
                nc.vector.scalar_tensor_tensor(vi[:], vn, 1.0, vn,
                                               op.is_lt, op.mult)
            s2 = vis[:].rearrange("c n t -> c (n t)")
            nc.gpsimd.tensor_scalar(s2, s2, 1.0, None, op.is_ge)

            # f1 filter: one masked scan over (n,t)
            nc.vector.tensor_tensor_scan(f1s[:], mask1[:], s2, 0.0,
                                         op.mult, op.add)

            # v1 inputs: tmp1[c,n,k,t] = f1[c,n,t]*w1[k] + b1[k]
            f1b = f1s[:].rearrange("c (n t) -> c n t", n=NL) \
                .unsqueeze(2).broadcast_to((C, NL, 10, T))
            w1b = wpk[:, 0:10].unsqueeze(1).unsqueeze(3) \
                .broadcast_to((C, NL, 10, T))
            b1b = wpk[:, 10:20].unsqueeze(1).unsqueeze(3) \
                .broadcast_to((C, NL, 10, T))
            t1v = tmp1[:].rearrange("c (n k t) -> c n k t", n=NL, k=10)
            nc.vector.tensor_tensor(t1v, f1b, w1b, op.mult)
            nc.vector.tensor_tensor(t1v, t1v, b1b, op.add)

            # ---------------- phase 3: v1 IF chain -----------------------
            t1r = tmp1[:].rearrange("c (n k t) -> c n k t", n=NL, k=10)
            for t in range(T):
                vn = v1s[:, :, :, t]
                nc.vector.tensor_tensor(vn, v1[:], t1r[:, :, :, t], op.add)
                nc.vector.scalar_tensor_tensor(v1[:], vn, 1.0, vn,
                                               op.is_lt, op.mult)
            s3 = v1s[:].rearrange("c n k t -> c (n k t)")
            nc.vector.tensor_scalar(s3, s3, 1.0, None, op.is_ge)

            # f2 filter: masked scan over all (n,k) segments
            nc.vector.tensor_tensor_scan(f2s[:], mask40[:], s3, 0.0,
                                         op.mult, op.add)

            # v2 inputs: m2[c,n,t,k] = f2[c,n,k,t]*w2[k]; red2 = sum_k + b2
            f2v = f2s[:].rearrange("c (n k t) -> c n k t", n=NL, k=10)
            w2b = wpk[:, 20:30].unsqueeze(1).unsqueeze(3) \
                .broadcast_to((C, NL, 10, T))
            # m2 stored (n,t,k) so the k-reduce is innermost; write it from
            # the (n,k,t) iteration via a transposed view
            m2v = m2[:].rearrange("c (n t k) -> c n t k", n=NL,
                                  t=T).transpose([0, 1, 3, 2])
            nc.vector.tensor_tensor(m2v, f2v, w2b, op.mult)
            nc.vector.tensor_reduce(
                red2[:], m2[:].rearrange("c (nt k) -> c nt k", k=10),
                mybir.AxisListType.X, op.add)

            # ---------------- phase 4: v2 IF chain -----------------------
            b2ap = wpk[:, 30:31]
            for t in range(T):
                vn = v2s[:, :, t]
                nc.vector.scalar_tensor_tensor(vn, v2[:], b2ap,
                                               red2[:, :, t], op.add, op.add)
                nc.vector.scalar_tensor_tensor(v2[:], vn, 1.0, vn,
                                               op.is_lt, op.mult)
            s4 = v2s[:].rearrange("c n t -> c (n t)")
            nc.vector.tensor_scalar(s4, s4, 1.0, None, op.is_ge)

            # fs filter + sum over channels (PE)
            nc.vector.tensor_tensor_scan(fss[:], mask1[:], s4, 0.0,
                                         op.mult, op.add)
            nc.tensor.matmul(sums_ps[:], wpk[:, 31:32], fss[:])
            sums_sb = pool.tile([1, NT], dt)
            nc.vector.tensor_scalar(sums_sb[:], sums_ps[:], 0.0, None,
                                    op.add)   # PSUM->SBUF (gpsimd can't PSUM)

            # ---------------- phase 5: vs IF chain -----------------------
            sums_v = sums_sb[:].rearrange("p (n t) -> p n t", n=NL)
            vsm = pool.tile([1, NL], dt)
            for t in range(T):
                vn = vss[:, :, t]
                nc.gpsimd.tensor_tensor(vn, vs[:], sums_v[:, :, t], op.add)
                nc.gpsimd.tensor_scalar(vsm[:], vn, 1.0, None, op.is_lt)
                nc.gpsimd.tensor_tensor(vs[:], vsm[:], vn, op.mult)
            hsv = vss[:].rearrange("p n t -> p (n t)")
            nc.gpsimd.tensor_scalar(hsv, hsv, 1.0, None, op.is_ge)

            # q0 inputs: sw0 outer h (PE, K=1) + sb0
            nc.tensor.matmul(q0_ps[:], srow[:, 0:32], hsv)
            nc.vector.tensor_scalar(tq0[:], q0_ps[:], spk[:, 1:2], None,
                                    op.add)

            # ---------------- phase 6: q0 IF chain -----------------------
            tq0v = tq0[:].rearrange("p (n t) -> p n t", n=NL)
            q0m = pool.tile([32, NL], dt)
            for t in range(T):
                vn = q0s[:, :, t]
                nc.gpsimd.tensor_tensor(vn, q0[:], tq0v[:, :, t], op.add)
                nc.gpsimd.tensor_scalar(q0m[:], vn, 1.0, None, op.is_lt)
                nc.gpsimd.tensor_tensor(q0[:], q0m[:], vn, op.mult)
            s5 = q0s[:].rearrange("p n t -> p (n t)")
            nc.gpsimd.tensor_scalar(s5, s5, 1.0, None, op.is_ge)

            # g1 filter + q1 inputs (PE 32x32) + sb1
            nc.vector.tensor_tensor_scan(g1s[:], mask1[0:32, :], s5, 0.0,
                                         op.mult, op.add)
            nc.tensor.matmul(q1_ps[:], spk[:, 3:35], g1s[:])
            nc.vector.tensor_scalar(tq1[:], q1_ps[:], spk[:, 2:3], None,
                                    op.add)

            # ---------------- phase 7: q1 IF chain -----------------------
            tq1v = tq1[:].rearrange("p (n t) -> p n t", n=NL)
            for t in range(T):
                vn = q1s[:, :, t]
                nc.vector.tensor_tensor(vn, q1[:], tq1v[:, :, t], op.add)
                nc.vector.scalar_tensor_tensor(q1[:], vn, 1.0, vn,
                                               op.is_lt, op.mult)
            s6 = q1s[:].rearrange("p n t -> p (n t)")
            nc.gpsimd.tensor_scalar(s6, s6, 1.0, None, op.is_ge)

            # g2 filter + q2 contributions (PE 32x1) + sb2, then cumsum
            nc.vector.tensor_tensor_scan(g2s[:], mask1[0:32, :], s6, 0.0,
                                         op.mult, op.add)
            nc.tensor.matmul(q2_ps[:], spk[:, 0:1], g2s[:])
            nc.vector.tensor_scalar(tq2[:], q2_ps[:], srow[:, 32:33], None,
                                    op.add)
            nc.vector.tensor_tensor_scan(q2s[:], cmask[:], tq2[:], 0.0,
                                         op.mult, op.add)

            # output: q2s[0, n*T+t] -> out[n, t] (contiguous)
            nc.sync.dma_start(
                out_d[:].rearrange("n t -> (n t)").unsqueeze(0), q2s[:])

    nc.compile()
    return nc, out_d.name


def _prep_core_inputs(x, w1, b1, w2, b2, sw0, sb0, sw1, sb1, sw2, sb2,
                      core):
    """Host-side marshalling of one core's shard into device layouts."""
    n0 = core * NL
    inv = F32(1.0) / TAU
    xs = x[:, n0:n0 + NL]                      # (T, NL, 2, C)
    x0 = np.ascontiguousarray(np.moveaxis(xs[:, :, 0, :], [0, 1, 2],
                                          [2, 1, 0]))   # (C, NL, T)
    x1 = np.ascontiguousarray(np.moveaxis(xs[:, :, 1, :], [0, 1, 2],
                                          [2, 1, 0]))
    x0R = np.zeros((C, NL, 128), F32)
    x0R[:, :, :T] = x0[:, :, ::-1] * inv       # x0R[...,tau']=x0[63-tau']/tau
    x1P = np.zeros((C, NL, 128), F32)
    x1P[:, :, 63:127] = x1 * inv               # x1P[...,tau]=x1[tau-63]/tau

    dist = np.arange(D) - D // 2
    kint = (1.0 / (1.0 - np.exp(-np.abs(dist) / 2.0)))
    kint[D // 2] = 1.0 / (1.0 - np.exp(-1.0 / 2.0))
    kint = np.broadcast_to(kint.astype(F32), (C, D)).copy()

    wpk = np.zeros((C, 32), F32)
    wpk[:, 0:10] = w1[:, 0]
    wpk[:, 10:20] = b1
    wpk[:, 20:30] = w2[0, :]
    wpk[:, 30] = b2[0]
    wpk[:, 31] = 1.0

    spk = np.zeros((32, 35), F32)
    spk[:, 0] = sw2[0, :]
    spk[:, 1] = sb0
    spk[:, 2] = sb1
    spk[:, 3:35] = sw1.T
    srow = np.zeros((1, 33), F32)
    srow[0, 0:32] = sw0[:, 0]
    srow[0, 32] = sb2[0]

    return {
        "x0r": x0R, "x1p": x1P, "kint": kint, "wpk": wpk, "spk": spk,
        "srow": srow,
    }


def kernel(x, w1, b1, w2, b2, sw0, sb0, sw1, sb1, sw2, sb2):
    from concourse.bass_utils import run_bass_kernel_spmd

    if "prog" not in _cache:
        _cache["prog"] = _build_program()
    nc, out_name = _cache["prog"]

    args = (x.astype(F32), w1, b1, w2, b2, sw0, sb0, sw1, sb1, sw2, sb2)
    in_maps = [_prep_core_inputs(*args, core) for core in range(NCORES)]
    res = run_bass_kernel_spmd(nc, in_maps, core_ids=list(range(NCORES)))
    out = np.concatenate([r[out_name] for r in res.results], axis=0)
    # device layout is (N, T); reference returns (T, N, 1)
    return np.ascontiguousarray(out.T)[:, :, None].astype(F32)


if __name__ == "__main__":
    d = np.load("/tmp/inputs.npz")
    out = kernel(**{k: d[k] for k in d.files})
    print("kernel out", out.shape, float(np.abs(out).max()))



# revision 2
# speedup vs baseline: 1.0365x; 1.0365x over previous
"""Trainium2 Bass kernel for nn_L2Net (Jeffress coincidence-detector SNN).

Pipelined redesign of the baseline:
  - 8-step chunks flow through a 17-slot software pipeline; every engine
    works concurrently (DVE + Pool split the serial LIF chains, Act does
    all spike extraction via Sign, PE does the cross-unit matmuls).
  - Jeffress LIF: u built per-step as one fp16 2x-mode tensor_tensor; the
    2-op/step membrane chain is split across DVE (cols 0:JA) and Pool
    (cols JA:256); pre-reset membranes stream to SBUF.
  - Spikes: Act Sign(v-1) = 2s-1; every consumer folds the (x+1)/2
    correction into an existing affine (tensor_scalar) or matmul fixup.
  - zc = sum_d kint*s via fp16 2x multiply + split reduce (DVE/Pool),
    with the Sign offset cancelled by a device-computed kint rowsum.
  - Downstream chains run as two skewed "stacks" sharing one update op
    pair per step: head [C, 48] = (vi, v1 x10, v2), tail [32, 12] =
    (vs, q0, q1); inputs are staged at skewed rows by per-chunk assembly
    (filters = masked scans with a 1-element carry, which commute with
    the linear maps; PE matmuls; bias fixes).

Slot s emission order (all per-chunk):   extractions (rows of slot s-1)
-> jeffress spikes/zc of chunk s-1 -> assemblies (f1/tmp1, m2/y2, vs,
q0, q1, q2) -> interleaved per-step ops: u, jeff DVE/Pool, head stack,
tail stack.
"""
import os
import sys

import numpy as np

sys.path.insert(0, "/opt/trn_rl_repo")

T, N, C, D = 64, 32, 128, 64
NCORES = 8
NL = N // NCORES          # samples per core
TAU = np.float32(20.0)    # jeffress LIF tau
F32 = np.float32
F16 = np.float16

CH = 8                    # steps per chunk
NCH = T // CH             # 8 chunks
JA = 128                  # jeffress chain cols on DVE (rest on Pool)

HSK_VI, HSK_V1, HSK_V2 = 16, 24, 32      # head-stack row offsets (t=0 row)
HROWS = HSK_V2 + T + 8                   # 104 rows
TSK_VS, TSK_Q0, TSK_Q1 = 0, 8, 16        # tail-stack row offsets
TROWS = TSK_Q1 + T + 8                   # 88

_cache = {}


def _build_program():
    import concourse.bass as bass
    import concourse.bacc as bacc
    import concourse.mybir as mybir
    import concourse.tile as tile

    dt32 = mybir.dt.float32
    dt16 = mybir.dt.float16
    op = mybir.AluOpType
    AF = mybir.ActivationFunctionType
    AX = mybir.AxisListType.X

    nc = bacc.Bacc("TRN2", target_bir_lowering=False, debug=False,
                   num_devices=NCORES)

    # ---------------- DRAM I/O ----------------
    xa_d = nc.dram_tensor("xa", [C, NL, 128], dt16, kind="ExternalInput")
    xb_d = nc.dram_tensor("xb", [C, NL, 128], dt16, kind="ExternalInput")
    kch_d = nc.dram_tensor("kch", [C, CH * NL * D], dt16,
                           kind="ExternalInput")
    # fp16 weight planes laid out (t8, n, k): [C, CH*NL*10]
    w1b_d = nc.dram_tensor("w1b", [C, CH * NL * 10], dt16,
                           kind="ExternalInput")
    b1b_d = nc.dram_tensor("b1b", [C, CH * NL * 10], dt16,
                           kind="ExternalInput")
    w2b_d = nc.dram_tensor("w2b", [C, CH * NL * 10], dt16,
                           kind="ExternalInput")
    cpk_d = nc.dram_tensor("cpk", [C, 4], dt32, kind="ExternalInput")
    spk_d = nc.dram_tensor("spk", [32, 40], dt32, kind="ExternalInput")
    srow_d = nc.dram_tensor("srow", [1, 40], dt32, kind="ExternalInput")
    onec_d = nc.dram_tensor("onec", [C, 2], dt16, kind="ExternalInput")
    out_d = nc.dram_tensor("out", [NL, T], dt32, kind="ExternalOutput")

    NKCH = NL * 10 * CH      # 320
    NTCH = NL * CH           # 32

    with tile.TileContext(nc) as tc:
        with (
            tc.tile_pool(name="pool", bufs=1) as pool,
            tc.tile_pool(name="psum", bufs=1, space="PSUM") as psum,
        ):
            # ---- constant / input tiles ----
            xa = pool.tile([C, NL, 128], dt16, name="xa")
            xb = pool.tile([C, NL, 128], dt16, name="xb")
            kch = pool.tile([C, CH, NL, D], dt16, name="kch")
            w1b = pool.tile([C, CH, NL, 10], dt16, name="w1b")
            b1b = pool.tile([C, CH, NL, 10], dt16, name="b1b")
            w2b = pool.tile([C, CH, NL, 10], dt16, name="w2b")
            cpk = pool.tile([C, 4], dt32, name="cpk")
            spk = pool.tile([32, 40], dt32, name="spk")
            srow = pool.tile([1, 40], dt32, name="srow")
            onec = pool.tile([C, 2], dt16, name="onec")
            for tl, dr in ((xa, xa_d), (xb, xb_d), (kch, kch_d),
                           (w1b, w1b_d), (b1b, b1b_d), (w2b, w2b_d),
                           (cpk, cpk_d), (spk, spk_d), (srow, srow_d),
                           (onec, onec_d)):
                nc.sync.dma_start(tl[:], dr[:])

            neg1 = pool.tile([C, 1], dt32, name="neg1")
            nc.vector.memset(neg1[:], -1.0)
            neg1t = pool.tile([32, 1], dt32, name="neg1t")
            nc.vector.memset(neg1t[:], -1.0)

            # device-exact Ktot via the SAME fp16 pairwise tree as the
            # per-chunk o-reduce below (negation commutes with add exactly,
            # so sgn=-1 rows cancel ko bit-exactly)
            kt1 = pool.tile([C, 32], dt16, name="kt1")
            kt2 = pool.tile([C, 16], dt16, name="kt2")
            kt3 = pool.tile([C, 8], dt16, name="kt3")
            kt4 = pool.tile([C, 4], dt16, name="kt4")
            krow = kch[:, 0, 0, :]
            nc.vector.tensor_tensor(kt1[:], krow[:, 0:32], krow[:, 32:64],
                                    op.add)
            nc.vector.tensor_tensor(kt2[:], kt1[:, 0:16], kt1[:, 16:32],
                                    op.add)
            nc.vector.tensor_tensor(kt3[:], kt2[:, 0:8], kt2[:, 8:16],
                                    op.add)
            nc.vector.tensor_tensor(kt4[:], kt3[:, 0:4], kt3[:, 4:8],
                                    op.add)
            ko = pool.tile([C, 1], dt32, name="ko")
            nc.vector.tensor_reduce(
                ko[:], kt4[:].rearrange("c d -> c () d"), AX, op.add)
            ko2 = pool.tile([C, 1], dt32, name="ko2")
            nc.vector.tensor_scalar(ko2[:], ko[:], 0.5, None, op.mult)
            st1 = pool.tile([C, 2, CH, NL, 32], dt16, name="st1")
            st2 = pool.tile([C, 2, CH, NL, 16], dt16, name="st2")
            st3 = pool.tile([C, 2, CH, NL, 8], dt16, name="st3")
            st4 = pool.tile([C, 2, CH, NL, 4], dt16, name="st4")

            # ---- jeffress state ----
            vj = pool.tile([C, NL, D], dt32, name="vj")
            vjs = pool.tile([C, T, NL, D], dt32, name="vjs")
            nc.vector.memset(vj[:], 0.0)

            sgnj = pool.tile([C, 2, CH, NL, D], dt16, name="sgnj")
            skr = pool.tile([C, 2, CH, NL, D], dt16, name="skr")

            # ---- head stack (vi 0:4 | v1 4:44 (n,k) | v2 44:48) ----
            HW = 48
            hv = pool.tile([C, HW], dt32, name="hv")
            hstg = pool.tile([C, HROWS, HW], dt32, name="hstg")
            hvn = pool.tile([C, HROWS, HW], dt32, name="hvn")
            nc.vector.memset(hv[:], 0.0)
            # only rows read by stack iters but never written by assembly
            nc.gpsimd.memset(hstg[:, 80:96, 0:4], 0.0)     # vi after t=63
            nc.gpsimd.memset(hstg[:, 16:24, 4:44], 0.0)    # v1 warmup
            nc.gpsimd.memset(hstg[:, 88:96, 4:44], 0.0)    # v1 drain
            nc.gpsimd.memset(hstg[:, 16:32, 44:48], 0.0)   # v2 warmup
            sgnh = pool.tile([C, 2, CH, HW], dt16, name="sgnh")

            # ---- tail stack (vs 0:4 (p0) | q0 4:8 | q1 8:12) ----
            TW = 12
            tv = pool.tile([32, TW], dt32, name="tv")
            tstg = pool.tile([32, TROWS, TW], dt32, name="tstg")
            tvn = pool.tile([32, TROWS, TW], dt32, name="tvn")
            nc.vector.memset(tv[:], 0.0)
            nc.gpsimd.memset(tstg[:, :, 0:4], 0.0)         # vs: p1:32 never
            nc.gpsimd.memset(tstg[:, 64:80, 0:4], 0.0)     # written, + drain
            nc.gpsimd.memset(tstg[:, 0:8, 4:8], 0.0)       # q0 warmup
            nc.gpsimd.memset(tstg[:, 72:80, 4:8], 0.0)     # q0 drain
            nc.gpsimd.memset(tstg[:, 0:16, 8:12], 0.0)     # q1 warmup
            sgnt = pool.tile([32, 2, CH, TW], dt32, name="sgnt")

            # ---- filter scan workspaces (with 1-element carry) ----
            def scanws(nm, p, maskval=0.5):
                si = pool.tile([p, NL, CH + 1], dt32, name=nm + "_in")
                so = pool.tile([p, NL, CH + 1], dt32, name=nm + "_out")
                sm = pool.tile([p, NL, CH + 1], dt32, name=nm + "_m")
                nc.vector.memset(si[:], 0.0)
                nc.vector.memset(so[:], 0.0)
                nc.vector.memset(sm[:], maskval)
                nc.vector.memset(sm[:, :, 0:1], 0.0)
                return si, so, sm

            f1i, f1o, f1m = scanws("f1", C)
            y2i, y2o, y2m = scanws("y2", C)
            vsi, vso, vsm = scanws("vs", 1)
            g1i, g1o, g1m = scanws("g1", 32)
            g2i, g2o, g2m = scanws("g2", 1)
            q2i, q2o, q2m = scanws("q2", 1, maskval=1.0)

            tmsk = pool.tile([32, TW], dt32, name="tmsk")
            c64h = pool.tile([1, 1], dt32, name="c64h")
            nc.vector.memset(c64h[:], float(C) / 2.0)
            tmp1 = pool.tile([C, 2, CH, NL, 10], dt16, name="tmp1")
            m2 = pool.tile([C, 2, CH, NL, 10], dt16, name="m2")
            m2r = pool.tile([C, 2, NL, CH], dt32, name="m2r")

            outs = pool.tile([1, NL, T], dt32, name="outs")

            ps_vs = psum.tile([1, 2, NTCH], dt32, name="ps_vs")
            ps_q0 = psum.tile([32, 2, NTCH], dt32, name="ps_q0")
            ps_q1 = psum.tile([32, 2, NTCH], dt32, name="ps_q1")
            ps_q2 = psum.tile([1, 2, NTCH], dt32, name="ps_q2")

            dec = float(F32(1.0) - F32(1.0) / TAU)     # 0.95

            # ============ pipeline ============
            NSLOT = NCH + 9
            for s in range(NSLOT):
                db = s % 2

                # ---- spike extractions for rows of slot s-1 ----
                hr = (s - 1) * CH
                if HSK_VI <= hr < HSK_V2 + T:
                    nc.scalar.activation(
                        sgnh[:, db].rearrange("c a w -> c (a w)"),
                        hvn[:, hr:hr + CH].rearrange("c a w -> c (a w)"),
                        AF.Sign, bias=neg1[:], scale=1.0)
                trx = (s - 7) * CH
                if 0 <= trx < TSK_Q1 + T:
                    nc.scalar.activation(
                        sgnt[:, db].rearrange("p a w -> p (a w)"),
                        tvn[:, trx:trx + CH].rearrange("p a w -> p (a w)"),
                        AF.Sign, bias=neg1t[:], scale=1.0)

                # ---- jeffress spikes + zc for chunk s-1 ----
                c1 = s - 1
                if 0 <= c1 < NCH:
                    blk = vjs[:, c1 * CH:(c1 + 1) * CH]
                    nc.scalar.activation(
                        sgnj[:, db].rearrange("c a n d -> c (a n d)"),
                        blk.rearrange("c a n d -> c (a n d)"),
                        AF.Sign, bias=neg1[:], scale=1.0)
                    nc.vector.tensor_tensor(
                        skr[:, db].rearrange("c a n d -> c (a n d)"),
                        sgnj[:, db].rearrange("c a n d -> c (a n d)"),
                        kch[:].rearrange("c a n d -> c (a n d)"), op.mult)
                    stg = hstg[:, HSK_VI + c1 * CH:HSK_VI + (c1 + 1) * CH,
                               0:4]
                    sk = skr[:, db]
                    nc.vector.tensor_tensor(st1[:, db], sk[:, :, :, 0:32],
                                            sk[:, :, :, 32:64], op.add)
                    nc.vector.tensor_tensor(st2[:, db],
                                            st1[:, db, :, :, 0:16],
                                            st1[:, db, :, :, 16:32], op.add)
                    nc.vector.tensor_tensor(st3[:, db],
                                            st2[:, db, :, :, 0:8],
                                            st2[:, db, :, :, 8:16], op.add)
                    nc.vector.tensor_tensor(st4[:, db],
                                            st3[:, db, :, :, 0:4],
                                            st3[:, db, :, :, 4:8], op.add)
                    nc.vector.tensor_reduce(stg, st4[:, db], AX, op.add)
                    # zc = (o + ko)/2 on Act: identity(0.5*x + ko/2)
                    nc.scalar.activation(stg, stg, AF.Identity,
                                         bias=ko2[:], scale=0.5)

                # ---- f1 scan + tmp1 (vi-sgn chunk s-3) ----
                cf1 = s - 3
                if 0 <= cf1 < NCH:
                    svi = sgnh[:, db, :, 0:4]            # [C, CH, NL]
                    nc.gpsimd.tensor_scalar(
                        f1i[:, :, 1:9].transpose([0, 2, 1]), svi, 1.0,
                        0.5, op.add, op.mult)
                    nc.gpsimd.tensor_scalar(f1i[:, :, 0:1], f1o[:, :, 8:9], 0.0, None, op.add)
                    nc.vector.tensor_tensor_scan(
                        f1o[:].rearrange("p n s -> p (n s)"),
                        f1m[:].rearrange("p n s -> p (n s)"),
                        f1i[:].rearrange("p n s -> p (n s)"),
                        0.0, op.mult, op.add)
                    # tmp1[c,a,n,k] = f1[c,n,a]*w1b ; stage = tmp1 + b1b
                    f1x = f1o[:, :, 1:9].transpose([0, 2, 1]) \
                        .unsqueeze(3).broadcast_to((C, CH, NL, 10))
                    nc.gpsimd.tensor_tensor(tmp1[:, db], f1x, w1b[:],
                                            op.mult)
                    stg1 = hstg[:, HSK_V1 + cf1 * CH:
                                HSK_V1 + (cf1 + 1) * CH, 4:44]
                    nc.gpsimd.tensor_tensor(
                        stg1.rearrange("c a (n k) -> c a n k", n=NL),
                        tmp1[:, db], b1b[:], op.add)

                # ---- m2 + y2 scan (v1-sgn chunk s-4) ----
                cm2 = s - 4
                if 0 <= cm2 < NCH:
                    sv1 = sgnh[:, db, :, 4:44]           # [C, CH, 40]
                    nc.gpsimd.tensor_tensor(
                        m2[:, db].rearrange("c a n k -> c a (n k)"),
                        sv1, w2b[:].rearrange("c a n k -> c a (n k)"),
                        op.mult)
                    nc.vector.tensor_reduce(
                        m2r[:, db], m2[:, db].transpose([0, 2, 1, 3]),
                        AX, op.add)
                    # y2 = (m2r + sum_w2)/2 -> scan -> v2 stage (+b2)
                    nc.gpsimd.tensor_scalar(
                        y2i[:, :, 1:9], m2r[:, db], cpk[:, 0:1], 0.5,
                        op.add, op.mult)
                    nc.gpsimd.tensor_scalar(y2i[:, :, 0:1], y2o[:, :, 8:9], 0.0, None, op.add)
                    nc.vector.tensor_tensor_scan(
                        y2o[:].rearrange("p n s -> p (n s)"),
                        y2m[:].rearrange("p n s -> p (n s)"),
                        y2i[:].rearrange("p n s -> p (n s)"),
                        0.0, op.mult, op.add)
                    stg2 = hstg[:, HSK_V2 + cm2 * CH:
                                HSK_V2 + (cm2 + 1) * CH, 44:48]
                    nc.gpsimd.tensor_scalar(
                        stg2, y2o[:, :, 1:9].transpose([0, 2, 1]),
                        cpk[:, 1:2], None, op.add)

                # ---- vs input (v2-sgn chunk s-5, PE C-sum) ----
                cvs = s - 5
                if 0 <= cvs < NCH:
                    sv2 = sgnh[:, db, :, 44:48]          # [C, CH, NL]
                    nc.tensor.matmul(
                        ps_vs[:, db], onec[:, 0:1],
                        sv2)
                    nc.scalar.activation(
                        vsi[:, :, 1:9],
                        ps_vs[:, db].rearrange("p (a n) -> p n a", a=CH),
                        AF.Identity, bias=c64h[:], scale=0.5)
                    nc.gpsimd.tensor_scalar(vsi[:, :, 0:1], vso[:, :, 8:9], 0.0, None, op.add)
                    nc.vector.tensor_tensor_scan(
                        vso[:].rearrange("p n s -> p (n s)"),
                        vsm[:].rearrange("p n s -> p (n s)"),
                        vsi[:].rearrange("p n s -> p (n s)"),
                        0.0, op.mult, op.add)
                    stgv = tstg[0:1, TSK_VS + cvs * CH:
                                TSK_VS + (cvs + 1) * CH, 0:4]
                    nc.gpsimd.tensor_scalar(
                        stgv, vso[:, :, 1:9].transpose([0, 2, 1]),
                        0.0, None, op.add)

                # ---- q0 input (vs-sgn chunk s-7, PE outer) ----
                cq0 = s - 7
                if 0 <= cq0 < NCH:
                    hs = sgnt[0:1, db, :, 0:4]           # [1, CH, NL]
                    nc.tensor.matmul(
                        ps_q0[:, db], srow[:, 0:32],
                        hs)
                    stq0 = tstg[:, TSK_Q0 + cq0 * CH:
                                TSK_Q0 + (cq0 + 1) * CH, 4:8]
                    nc.scalar.activation(
                        stq0,
                        ps_q0[:, db].rearrange("p (a n) -> p a n", a=CH),
                        AF.Identity, bias=spk[:, 36:37], scale=0.5)

                # ---- q1 input (q0-sgn chunk s-8, PE 32x32) ----
                cq1 = s - 8
                if 0 <= cq1 < NCH:
                    sq0 = sgnt[:, db, :, 4:8]            # [32, CH, NL]
                    nc.tensor.matmul(
                        ps_q1[:, db], spk[:, 0:32],
                        sq0)
                    nc.scalar.activation(
                        g1i[:, :, 1:9],
                        ps_q1[:, db].rearrange("p (a n) -> p n a", a=CH),
                        AF.Identity, bias=spk[:, 37:38], scale=0.5)
                    nc.gpsimd.tensor_scalar(g1i[:, :, 0:1], g1o[:, :, 8:9], 0.0, None, op.add)
                    nc.vector.tensor_tensor_scan(
                        g1o[:].rearrange("p n s -> p (n s)"),
                        g1m[:].rearrange("p n s -> p (n s)"),
                        g1i[:].rearrange("p n s -> p (n s)"),
                        0.0, op.mult, op.add)
                    stq1 = tstg[:, TSK_Q1 + cq1 * CH:
                                TSK_Q1 + (cq1 + 1) * CH, 8:12]
                    nc.scalar.activation(
                        stq1, g1o[:, :, 1:9].transpose([0, 2, 1]),
                        AF.Identity, bias=spk[:, 34:35], scale=1.0)

                # ---- q2 (q1-sgn chunk s-9, PE 32->1, 2 scans) ----
                cq2 = s - 9
                if 0 <= cq2 < NCH:
                    sq1 = sgnt[:, db, :, 8:12]           # [32, CH, NL]
                    nc.tensor.matmul(
                        ps_q2[:, db], spk[:, 35:36],
                        sq1)
                    nc.scalar.activation(
                        g2i[:, :, 1:9],
                        ps_q2[:, db].rearrange("p (a n) -> p n a", a=CH),
                        AF.Identity, bias=srow[:, 34:35], scale=0.5)
                    nc.gpsimd.tensor_scalar(g2i[:, :, 0:1], g2o[:, :, 8:9], 0.0, None, op.add)
                    nc.vector.tensor_tensor_scan(
                        g2o[:].rearrange("p n s -> p (n s)"),
                        g2m[:].rearrange("p n s -> p (n s)"),
                        g2i[:].rearrange("p n s -> p (n s)"),
                        0.0, op.mult, op.add)
                    nc.gpsimd.tensor_scalar(
                        q2i[:, :, 1:9], g2o[:, :, 1:9],
                        srow[:, 33:34], None, op.add)
                    nc.gpsimd.tensor_scalar(q2i[:, :, 0:1], q2o[:, :, 8:9], 0.0, None, op.add)
                    nc.vector.tensor_tensor_scan(
                        q2o[:].rearrange("p n s -> p (n s)"),
                        q2m[:].rearrange("p n s -> p (n s)"),
                        q2i[:].rearrange("p n s -> p (n s)"),
                        0.0, op.mult, op.add)
                    nc.scalar.copy(
                        outs[:, :, cq2 * CH:(cq2 + 1) * CH],
                        q2o[:, :, 1:9])

                # ---- interleaved per-step serial chains ----
                jeff = s < NCH
                r0 = s * CH
                head = HSK_VI <= r0 < HSK_V2 + T
                tr0 = (s - 6) * CH
                tail = 0 <= tr0 < TSK_Q1 + T
                for i in range(CH):
                    t = s * CH + i
                    if jeff:
                        u_t = pool.tile([C, NL, D], dt16, name="u_t",
                                        tag="u", bufs=3)
                        nc.gpsimd.tensor_tensor(
                            u_t[:], xa[:, :, 63 - t:127 - t],
                            xb[:, :, t:t + 64], op.add)
                        vna = vjs[:, t].rearrange("c n d -> c (n d)")
                        vjf = vj[:].rearrange("c n d -> c (n d)")
                        uf = u_t[:].rearrange("c n d -> c (n d)")
                        nc.vector.scalar_tensor_tensor(
                            vna, vjf, dec, uf, op.mult, op.add)
                    if head:
                        r = r0 + i
                        nc.vector.tensor_tensor(
                            hvn[:, r], hv[:], hstg[:, r], op.add)
                    if jeff:
                        nc.vector.scalar_tensor_tensor(
                            vjf, vna, 1.0, vna, op.is_lt, op.mult)
                    if tail:
                        rt = tr0 + i
                        nc.vector.tensor_tensor(
                            tvn[:, rt], tv[:], tstg[:, rt], op.add)
                    if head:
                        nc.vector.scalar_tensor_tensor(
                            hv[:], hvn[:, r], 1.0, hvn[:, r],
                            op.is_lt, op.mult)
                    if tail:
                        nc.vector.scalar_tensor_tensor(
                            tv[:], tvn[:, rt], 1.0, tvn[:, rt],
                            op.is_lt, op.mult)

            nc.sync.dma_start(
                out_d[:].rearrange("n t -> (n t)").unsqueeze(0),
                outs[:].rearrange("p n t -> p (n t)"))

    nc.compile()
    return nc, out_d.name


def _prep_core_inputs(x, w1, b1, w2, b2, sw0, sb0, sw1, sb1, sw2, sb2,
                      core):
    """Host-side marshalling of one core's shard into device layouts."""
    n0 = core * NL
    inv = F32(1.0) / TAU
    xs = x[:, n0:n0 + NL]                      # (T, NL, 2, C)
    x0 = np.ascontiguousarray(np.moveaxis(xs[:, :, 0, :], [0, 1, 2],
                                          [2, 1, 0]))   # (C, NL, T)
    x1 = np.ascontiguousarray(np.moveaxis(xs[:, :, 1, :], [0, 1, 2],
                                          [2, 1, 0]))
    xa = np.zeros((C, NL, 128), F16)
    xa[:, :, :T] = (x0[:, :, ::-1] * inv).astype(F16)
    xb = np.zeros((C, NL, 128), F16)
    xb[:, :, 63:127] = (x1 * inv).astype(F16)

    dist = np.arange(D) - D // 2
    with np.errstate(divide="ignore"):
        kint = 1.0 / (1.0 - np.exp(-np.abs(dist) / 2.0))
    kint[D // 2] = 1.0 / (1.0 - np.exp(-1.0 / 2.0))
    kint16 = kint.astype(F16)
    kch = np.broadcast_to(kint16, (C, CH, NL, D)).reshape(C, -1).copy()

    # fp16 weight planes, laid out (t8, n, k)
    w1b = np.broadcast_to(w1[:, 0].astype(F16)[None, None, None, :],
                          (C, CH, NL, 10)).reshape(C, -1).copy()
    b1b = np.broadcast_to(b1.astype(F16)[None, None, None, :],
                          (C, CH, NL, 10)).reshape(C, -1).copy()
    w2b = np.broadcast_to(w2[0, :].astype(F16)[None, None, None, :],
                          (C, CH, NL, 10)).reshape(C, -1).copy()

    cpk = np.zeros((C, 4), F32)
    cpk[:, 0] = np.float16(w2[0, :]).astype(F32).sum()   # sum_k w2 (fp16)
    cpk[:, 1] = b2[0]

    spk = np.zeros((32, 40), F32)
    spk[:, 0:32] = sw1.T                       # stat for q1 matmul
    spk[:, 32] = sw0[:, 0] + 2.0 * sb0         # q0 comb
    spk[:, 33] = sw1.sum(axis=1)               # rowsum sw1 (g1 fix)
    spk[:, 34] = sb1
    spk[:, 35] = sw2[0, :]                     # stat for q2 matmul
    spk[:, 36] = (sw0[:, 0] + 2.0 * sb0) / 2.0
    spk[:, 37] = sw1.sum(axis=1) / 2.0

    srow = np.zeros((1, 40), F32)
    srow[0, 0:32] = sw0[:, 0]                  # stat for q0 matmul
    srow[0, 32] = sw2[0, :].sum()              # g2 fix
    srow[0, 33] = sb2[0]
    srow[0, 34] = sw2[0, :].sum() / 2.0

    onec = np.ones((C, 2), F16)

    return {
        "xa": xa, "xb": xb, "kch": kch, "w1b": w1b, "b1b": b1b,
        "w2b": w2b, "cpk": cpk, "spk": spk, "srow": srow, "onec": onec,
    }


def kernel(x, w1, b1, w2, b2, sw0, sb0, sw1, sb1, sw2, sb2):
    from concourse.bass_utils import run_bass_kernel_spmd

    if "prog" not in _cache:
        _cache["prog"] = _build_program()
    nc, out_name = _cache["prog"]

    args = (x.astype(F32), w1, b1, w2, b2, sw0, sb0, sw1, sb1, sw2, sb2)
    in_maps = [_prep_core_inputs(*args, core) for core in range(NCORES)]
    res = run_bass_kernel_spmd(nc, in_maps, core_ids=list(range(NCORES)))
    out = np.concatenate([r[out_name] for r in res.results], axis=0)
    # device layout is (N, T); reference returns (T, N, 1)
    return np.ascontiguousarray(out.T)[:, :, None].astype(F32)


if __name__ == "__main__":
    d = np.load("/tmp/inputs.npz")
    out = kernel(**{k: d[k] for k in d.files})
    print("kernel out", out.shape, float(np.abs(out).max()))


# revision 3
# speedup vs baseline: 1.1395x; 1.0994x over previous
"""Trainium2 Bass kernel for nn_L2Net (Jeffress coincidence-detector SNN).

Pipelined redesign of the baseline:
  - 8-step chunks flow through a 17-slot software pipeline; every engine
    works concurrently (DVE + Pool split the serial LIF chains, Act does
    all spike extraction via Sign, PE does the cross-unit matmuls).
  - Jeffress LIF: u built per-step as one fp16 2x-mode tensor_tensor; the
    2-op/step membrane chain is split across DVE (cols 0:JA) and Pool
    (cols JA:256); pre-reset membranes stream to SBUF.
  - Spikes: Act Sign(v-1) = 2s-1; every consumer folds the (x+1)/2
    correction into an existing affine (tensor_scalar) or matmul fixup.
  - zc = sum_d kint*s via fp16 2x multiply + split reduce (DVE/Pool),
    with the Sign offset cancelled by a device-computed kint rowsum.
  - Downstream chains run as two skewed "stacks" sharing one update op
    pair per step: head [C, 48] = (vi, v1 x10, v2), tail [32, 12] =
    (vs, q0, q1); inputs are staged at skewed rows by per-chunk assembly
    (filters = masked scans with a 1-element carry, which commute with
    the linear maps; PE matmuls; bias fixes).

Slot s emission order (all per-chunk):   extractions (rows of slot s-1)
-> jeffress spikes/zc of chunk s-1 -> assemblies (f1/tmp1, m2/y2, vs,
q0, q1, q2) -> interleaved per-step ops: u, jeff DVE/Pool, head stack,
tail stack.
"""
import os
import sys

import numpy as np

sys.path.insert(0, "/opt/trn_rl_repo")

T, N, C, D = 64, 32, 128, 64
NCORES = 8
NL = N // NCORES          # samples per core
TAU = np.float32(20.0)    # jeffress LIF tau
F32 = np.float32
F16 = np.float16

CH = 8                    # steps per chunk
NCH = T // CH             # 8 chunks
JA = 128                  # jeffress chain cols on DVE (rest on Pool)

HSK_VI, HSK_V1, HSK_V2 = 16, 24, 32      # head-stack row offsets (t=0 row)
HROWS = HSK_V2 + T + 8                   # 104 rows
TSK_VS, TSK_Q0, TSK_Q1 = 0, 8, 16        # tail-stack row offsets
TROWS = TSK_Q1 + T + 8                   # 88

_cache = {}


def _build_program():
    import concourse.bass as bass
    import concourse.bacc as bacc
    import concourse.mybir as mybir
    import concourse.tile as tile

    dt32 = mybir.dt.float32
    dt16 = mybir.dt.float16
    op = mybir.AluOpType
    AF = mybir.ActivationFunctionType
    AX = mybir.AxisListType.X

    nc = bacc.Bacc("TRN2", target_bir_lowering=False, debug=False,
                   num_devices=NCORES)

    # ---------------- DRAM I/O ----------------
    xa_d = nc.dram_tensor("xa", [C, NL, 128], dt16, kind="ExternalInput")
    xb_d = nc.dram_tensor("xb", [C, NL, 128], dt16, kind="ExternalInput")
    kch_d = nc.dram_tensor("kch", [C, CH * NL * D], dt16,
                           kind="ExternalInput")
    # fp16 weight planes laid out (t8, n, k): [C, CH*NL*10]
    w1b_d = nc.dram_tensor("w1b", [C, CH * NL * 10], dt16,
                           kind="ExternalInput")
    b1b_d = nc.dram_tensor("b1b", [C, CH * NL * 10], dt16,
                           kind="ExternalInput")
    w2b_d = nc.dram_tensor("w2b", [C, CH * NL * 10], dt16,
                           kind="ExternalInput")
    cpk_d = nc.dram_tensor("cpk", [C, 4], dt32, kind="ExternalInput")
    spk_d = nc.dram_tensor("spk", [32, 40], dt32, kind="ExternalInput")
    srow_d = nc.dram_tensor("srow", [1, 40], dt32, kind="ExternalInput")
    onec_d = nc.dram_tensor("onec", [C, 2], dt16, kind="ExternalInput")
    out_d = nc.dram_tensor("out", [NL, T], dt32, kind="ExternalOutput")

    NKCH = NL * 10 * CH      # 320
    NTCH = NL * CH           # 32

    with tile.TileContext(nc) as tc:
        with (
            tc.tile_pool(name="pool", bufs=1) as pool,
            tc.tile_pool(name="psum", bufs=1, space="PSUM") as psum,
        ):
            # ---- constant / input tiles ----
            xa = pool.tile([C, NL, 128], dt16, name="xa")
            xb = pool.tile([C, NL, 128], dt16, name="xb")
            kch = pool.tile([C, CH, NL, D], dt16, name="kch")
            w1b = pool.tile([C, CH, NL, 10], dt16, name="w1b")
            b1b = pool.tile([C, CH, NL, 10], dt16, name="b1b")
            w2b = pool.tile([C, CH, NL, 10], dt16, name="w2b")
            cpk = pool.tile([C, 4], dt32, name="cpk")
            spk = pool.tile([32, 40], dt32, name="spk")
            srow = pool.tile([1, 40], dt32, name="srow")
            onec = pool.tile([C, 2], dt16, name="onec")
            for tl, dr in ((xa, xa_d), (xb, xb_d), (kch, kch_d),
                           (w1b, w1b_d), (b1b, b1b_d), (w2b, w2b_d),
                           (cpk, cpk_d), (spk, spk_d), (srow, srow_d),
                           (onec, onec_d)):
                nc.sync.dma_start(tl[:], dr[:])

            neg1 = pool.tile([C, 1], dt32, name="neg1")
            nc.vector.memset(neg1[:], -1.0)
            neg1t = pool.tile([32, 1], dt32, name="neg1t")
            nc.vector.memset(neg1t[:], -1.0)

            # device-exact Ktot via the SAME fp16 pairwise tree as the
            # per-chunk o-reduce below (negation commutes with add exactly,
            # so sgn=-1 rows cancel ko bit-exactly)
            kt1 = pool.tile([C, 32], dt16, name="kt1")
            kt2 = pool.tile([C, 16], dt16, name="kt2")
            kt3 = pool.tile([C, 8], dt16, name="kt3")
            kt4 = pool.tile([C, 4], dt16, name="kt4")
            krow = kch[:, 0, 0, :]
            nc.vector.tensor_tensor(kt1[:], krow[:, 0:32], krow[:, 32:64],
                                    op.add)
            nc.vector.tensor_tensor(kt2[:], kt1[:, 0:16], kt1[:, 16:32],
                                    op.add)
            nc.vector.tensor_tensor(kt3[:], kt2[:, 0:8], kt2[:, 8:16],
                                    op.add)
            nc.vector.tensor_tensor(kt4[:], kt3[:, 0:4], kt3[:, 4:8],
                                    op.add)
            ko = pool.tile([C, 1], dt32, name="ko")
            nc.vector.tensor_reduce(
                ko[:], kt4[:].rearrange("c d -> c () d"), AX, op.add)
            ko2 = pool.tile([C, 1], dt32, name="ko2")
            nc.vector.tensor_scalar(ko2[:], ko[:], 0.5, None, op.mult)
            st1 = pool.tile([C, 2, CH, NL, 32], dt16, name="st1")
            st2 = pool.tile([C, 2, CH, NL, 16], dt16, name="st2")
            st3 = pool.tile([C, 2, CH, NL, 8], dt16, name="st3")
            st4 = pool.tile([C, 2, CH, NL, 4], dt16, name="st4")

            # ---- jeffress state ----
            vj = pool.tile([C, NL, D], dt32, name="vj")
            vjs = pool.tile([C, T, NL, D], dt32, name="vjs")
            nc.vector.memset(vj[:], 0.0)

            sgnj = pool.tile([C, 2, CH, NL, D], dt16, name="sgnj")
            skr = pool.tile([C, 2, CH, NL, D], dt16, name="skr")

            # ---- head stack (vi 0:4 | v1 4:44 (n,k) | v2 44:48) ----
            HW = 48
            hv = pool.tile([C, HW], dt32, name="hv")
            hstg = pool.tile([C, HROWS, HW], dt32, name="hstg")
            hvn = pool.tile([C, HROWS, HW], dt32, name="hvn")
            nc.vector.memset(hv[:], 0.0)
            # only rows read by stack iters but never written by assembly
            nc.gpsimd.memset(hstg[:, 80:96, 0:4], 0.0)     # vi after t=63
            nc.gpsimd.memset(hstg[:, 16:24, 4:44], 0.0)    # v1 warmup
            nc.gpsimd.memset(hstg[:, 88:96, 4:44], 0.0)    # v1 drain
            nc.gpsimd.memset(hstg[:, 16:32, 44:48], 0.0)   # v2 warmup
            sgnh = pool.tile([C, 2, CH, HW], dt16, name="sgnh")

            # ---- tail stack (vs 0:4 (p0) | q0 4:8 | q1 8:12) ----
            TW = 12
            tv = pool.tile([32, TW], dt32, name="tv")
            tstg = pool.tile([32, TROWS, TW], dt32, name="tstg")
            tvn = pool.tile([32, TROWS, TW], dt32, name="tvn")
            nc.vector.memset(tv[:], 0.0)
            nc.gpsimd.memset(tstg[:, :, 0:4], 0.0)         # vs: p1:32 never
            nc.gpsimd.memset(tstg[:, 64:80, 0:4], 0.0)     # written, + drain
            nc.gpsimd.memset(tstg[:, 0:8, 4:8], 0.0)       # q0 warmup
            nc.gpsimd.memset(tstg[:, 72:80, 4:8], 0.0)     # q0 drain
            nc.gpsimd.memset(tstg[:, 0:16, 8:12], 0.0)     # q1 warmup
            sgnt = pool.tile([32, 2, CH, TW], dt32, name="sgnt")

            # ---- filter scan workspaces (with 1-element carry) ----
            def scanws(nm, p, maskval=0.5):
                si = pool.tile([p, NL, CH + 1], dt32, name=nm + "_in")
                so = pool.tile([p, NL, CH + 1], dt32, name=nm + "_out")
                sm = pool.tile([p, NL, CH + 1], dt32, name=nm + "_m")
                nc.vector.memset(si[:], 0.0)
                nc.vector.memset(so[:], 0.0)
                nc.vector.memset(sm[:], maskval)
                nc.vector.memset(sm[:, :, 0:1], 0.0)
                return si, so, sm

            f1i, f1o, f1m = scanws("f1", C)
            y2i, y2o, y2m = scanws("y2", C)
            vsi, vso, vsm = scanws("vs", 1)
            g1i, g1o, g1m = scanws("g1", 32)
            g2i, g2o, g2m = scanws("g2", 1)
            q2i, q2o, q2m = scanws("q2", 1, maskval=1.0)

            tmsk = pool.tile([32, TW], dt32, name="tmsk")
            c64h = pool.tile([1, 1], dt32, name="c64h")
            nc.vector.memset(c64h[:], float(C) / 2.0)
            tmp1 = pool.tile([C, 2, CH, NL, 10], dt16, name="tmp1")
            m2 = pool.tile([C, 2, CH, NL, 10], dt16, name="m2")
            m2r = pool.tile([C, 2, NL, CH], dt32, name="m2r")

            outs = pool.tile([1, NL, T], dt32, name="outs")

            ps_vs = psum.tile([1, 2, NTCH], dt32, name="ps_vs")
            ps_q0 = psum.tile([32, 2, NTCH], dt32, name="ps_q0")
            ps_q1 = psum.tile([32, 2, NTCH], dt32, name="ps_q1")
            ps_q2 = psum.tile([1, 2, NTCH], dt32, name="ps_q2")

            dec = float(F32(1.0) - F32(1.0) / TAU)     # 0.95

            # ============ pipeline ============
            NSLOT = NCH + 9
            for s in range(NSLOT):
                db = s % 2

                # ---- spike extractions for rows of slot s-1 ----
                hr = (s - 1) * CH
                if HSK_VI <= hr < HSK_V2 + T:
                    nc.scalar.activation(
                        sgnh[:, db].rearrange("c a w -> c (a w)"),
                        hvn[:, hr:hr + CH].rearrange("c a w -> c (a w)"),
                        AF.Sign, bias=neg1[:], scale=1.0)
                trx = (s - 7) * CH
                if 0 <= trx < TSK_Q1 + T:
                    nc.scalar.activation(
                        sgnt[:, db].rearrange("p a w -> p (a w)"),
                        tvn[:, trx:trx + CH].rearrange("p a w -> p (a w)"),
                        AF.Sign, bias=neg1t[:], scale=1.0)

                # ---- jeffress spikes + zc for chunk s-1 ----
                c1 = s - 1
                if 0 <= c1 < NCH:
                    blk = vjs[:, c1 * CH:(c1 + 1) * CH]
                    nc.scalar.activation(
                        sgnj[:, db].rearrange("c a n d -> c (a n d)"),
                        blk.rearrange("c a n d -> c (a n d)"),
                        AF.Sign, bias=neg1[:], scale=1.0)
                    nc.vector.tensor_tensor(
                        skr[:, db].rearrange("c a n d -> c (a n d)"),
                        sgnj[:, db].rearrange("c a n d -> c (a n d)"),
                        kch[:].rearrange("c a n d -> c (a n d)"), op.mult)
                    stg = hstg[:, HSK_VI + c1 * CH:HSK_VI + (c1 + 1) * CH,
                               0:4]
                    sk = skr[:, db]
                    nc.vector.tensor_tensor(st1[:, db], sk[:, :, :, 0:32],
                                            sk[:, :, :, 32:64], op.add)
                    nc.vector.tensor_tensor(st2[:, db],
                                            st1[:, db, :, :, 0:16],
                                            st1[:, db, :, :, 16:32], op.add)
                    nc.vector.tensor_tensor(st3[:, db],
                                            st2[:, db, :, :, 0:8],
                                            st2[:, db, :, :, 8:16], op.add)
                    nc.vector.tensor_tensor(st4[:, db],
                                            st3[:, db, :, :, 0:4],
                                            st3[:, db, :, :, 4:8], op.add)
                    nc.vector.tensor_reduce(stg, st4[:, db], AX, op.add)
                    # zc = (o + ko)/2 on Act: identity(0.5*x + ko/2)
                    nc.scalar.activation(stg, stg, AF.Identity,
                                         bias=ko2[:], scale=0.5)

                # ---- f1 scan + tmp1 (vi-sgn chunk s-3) ----
                cf1 = s - 3
                if 0 <= cf1 < NCH:
                    svi = sgnh[:, db, :, 0:4]            # [C, CH, NL]
                    nc.gpsimd.tensor_scalar(
                        f1i[:, :, 1:9].transpose([0, 2, 1]), svi, 1.0,
                        0.5, op.add, op.mult)
                    nc.gpsimd.tensor_scalar(f1i[:, :, 0:1], f1o[:, :, 8:9], 0.0, None, op.add)
                    nc.vector.tensor_tensor_scan(
                        f1o[:].rearrange("p n s -> p (n s)"),
                        f1m[:].rearrange("p n s -> p (n s)"),
                        f1i[:].rearrange("p n s -> p (n s)"),
                        0.0, op.mult, op.add)
                    # tmp1[c,a,n,k] = f1[c,n,a]*w1b ; stage = tmp1 + b1b
                    f1x = f1o[:, :, 1:9].transpose([0, 2, 1]) \
                        .unsqueeze(3).broadcast_to((C, CH, NL, 10))
                    nc.gpsimd.tensor_tensor(tmp1[:, db], f1x, w1b[:],
                                            op.mult)
                    stg1 = hstg[:, HSK_V1 + cf1 * CH:
                                HSK_V1 + (cf1 + 1) * CH, 4:44]
                    nc.gpsimd.tensor_tensor(
                        stg1.rearrange("c a (n k) -> c a n k", n=NL),
                        tmp1[:, db], b1b[:], op.add)

                # ---- m2 + y2 scan (v1-sgn chunk s-4) ----
                cm2 = s - 4
                if 0 <= cm2 < NCH:
                    sv1 = sgnh[:, db, :, 4:44]           # [C, CH, 40]
                    nc.gpsimd.tensor_tensor(
                        m2[:, db].rearrange("c a n k -> c a (n k)"),
                        sv1, w2b[:].rearrange("c a n k -> c a (n k)"),
                        op.mult)
                    nc.vector.tensor_reduce(
                        m2r[:, db], m2[:, db].transpose([0, 2, 1, 3]),
                        AX, op.add)
                    # y2 = (m2r + sum_w2)/2 -> scan -> v2 stage (+b2)
                    nc.gpsimd.tensor_scalar(
                        y2i[:, :, 1:9], m2r[:, db], cpk[:, 0:1], 0.5,
                        op.add, op.mult)
                    nc.gpsimd.tensor_scalar(y2i[:, :, 0:1], y2o[:, :, 8:9], 0.0, None, op.add)
                    nc.vector.tensor_tensor_scan(
                        y2o[:].rearrange("p n s -> p (n s)"),
                        y2m[:].rearrange("p n s -> p (n s)"),
                        y2i[:].rearrange("p n s -> p (n s)"),
                        0.0, op.mult, op.add)
                    stg2 = hstg[:, HSK_V2 + cm2 * CH:
                                HSK_V2 + (cm2 + 1) * CH, 44:48]
                    nc.gpsimd.tensor_scalar(
                        stg2, y2o[:, :, 1:9].transpose([0, 2, 1]),
                        cpk[:, 1:2], None, op.add)

                # ---- vs input (v2-sgn chunk s-5, PE C-sum) ----
                cvs = s - 5
                if 0 <= cvs < NCH:
                    sv2 = sgnh[:, db, :, 44:48]          # [C, CH, NL]
                    nc.tensor.matmul(
                        ps_vs[:, db], onec[:, 0:1],
                        sv2)
                    nc.scalar.activation(
                        vsi[:, :, 1:9],
                        ps_vs[:, db].rearrange("p (a n) -> p n a", a=CH),
                        AF.Identity, bias=c64h[:], scale=0.5)
                    nc.gpsimd.tensor_scalar(vsi[:, :, 0:1], vso[:, :, 8:9], 0.0, None, op.add)
                    nc.vector.tensor_tensor_scan(
                        vso[:].rearrange("p n s -> p (n s)"),
                        vsm[:].rearrange("p n s -> p (n s)"),
                        vsi[:].rearrange("p n s -> p (n s)"),
                        0.0, op.mult, op.add)
                    stgv = tstg[0:1, TSK_VS + cvs * CH:
                                TSK_VS + (cvs + 1) * CH, 0:4]
                    nc.gpsimd.tensor_scalar(
                        stgv, vso[:, :, 1:9].transpose([0, 2, 1]),
                        0.0, None, op.add)

                # ---- q0 input (vs-sgn chunk s-7, PE outer) ----
                cq0 = s - 7
                if 0 <= cq0 < NCH:
                    hs = sgnt[0:1, db, :, 0:4]           # [1, CH, NL]
                    nc.tensor.matmul(
                        ps_q0[:, db], srow[:, 0:32],
                        hs)
                    stq0 = tstg[:, TSK_Q0 + cq0 * CH:
                                TSK_Q0 + (cq0 + 1) * CH, 4:8]
                    nc.scalar.activation(
                        stq0,
                        ps_q0[:, db].rearrange("p (a n) -> p a n", a=CH),
                        AF.Identity, bias=spk[:, 36:37], scale=0.5)

                # ---- q1 input (q0-sgn chunk s-8, PE 32x32) ----
                cq1 = s - 8
                if 0 <= cq1 < NCH:
                    sq0 = sgnt[:, db, :, 4:8]            # [32, CH, NL]
                    nc.tensor.matmul(
                        ps_q1[:, db], spk[:, 0:32],
                        sq0)
                    nc.scalar.activation(
                        g1i[:, :, 1:9],
                        ps_q1[:, db].rearrange("p (a n) -> p n a", a=CH),
                        AF.Identity, bias=spk[:, 37:38], scale=0.5)
                    nc.gpsimd.tensor_scalar(g1i[:, :, 0:1], g1o[:, :, 8:9], 0.0, None, op.add)
                    nc.vector.tensor_tensor_scan(
                        g1o[:].rearrange("p n s -> p (n s)"),
                        g1m[:].rearrange("p n s -> p (n s)"),
                        g1i[:].rearrange("p n s -> p (n s)"),
                        0.0, op.mult, op.add)
                    stq1 = tstg[:, TSK_Q1 + cq1 * CH:
                                TSK_Q1 + (cq1 + 1) * CH, 8:12]
                    nc.scalar.activation(
                        stq1, g1o[:, :, 1:9].transpose([0, 2, 1]),
                        AF.Identity, bias=spk[:, 34:35], scale=1.0)

                # ---- q2 (q1-sgn chunk s-9, PE 32->1, 2 scans) ----
                cq2 = s - 9
                if 0 <= cq2 < NCH:
                    sq1 = sgnt[:, db, :, 8:12]           # [32, CH, NL]
                    nc.tensor.matmul(
                        ps_q2[:, db], spk[:, 35:36],
                        sq1)
                    nc.scalar.activation(
                        g2i[:, :, 1:9],
                        ps_q2[:, db].rearrange("p (a n) -> p n a", a=CH),
                        AF.Identity, bias=srow[:, 34:35], scale=0.5)
                    nc.gpsimd.tensor_scalar(g2i[:, :, 0:1], g2o[:, :, 8:9], 0.0, None, op.add)
                    nc.vector.tensor_tensor_scan(
                        g2o[:].rearrange("p n s -> p (n s)"),
                        g2m[:].rearrange("p n s -> p (n s)"),
                        g2i[:].rearrange("p n s -> p (n s)"),
                        0.0, op.mult, op.add)
                    nc.gpsimd.tensor_scalar(
                        q2i[:, :, 1:9], g2o[:, :, 1:9],
                        srow[:, 33:34], None, op.add)
                    nc.gpsimd.tensor_scalar(q2i[:, :, 0:1], q2o[:, :, 8:9], 0.0, None, op.add)
                    nc.vector.tensor_tensor_scan(
                        q2o[:].rearrange("p n s -> p (n s)"),
                        q2m[:].rearrange("p n s -> p (n s)"),
                        q2i[:].rearrange("p n s -> p (n s)"),
                        0.0, op.mult, op.add)
                    nc.scalar.copy(
                        outs[:, :, cq2 * CH:(cq2 + 1) * CH],
                        q2o[:, :, 1:9])

                # ---- interleaved per-step serial chains ----
                jeff = s < NCH
                r0 = s * CH
                head = HSK_VI <= r0 < HSK_V2 + T
                tr0 = (s - 6) * CH
                tail = 0 <= tr0 < TSK_Q1 + T
                for i in range(CH):
                    t = s * CH + i
                    if jeff:
                        u_t = pool.tile([C, NL, D], dt16, name="u_t",
                                        tag="u", bufs=10)
                        nc.gpsimd.tensor_tensor(
                            u_t[:], xa[:, :, 63 - t:127 - t],
                            xb[:, :, t:t + 64], op.add)
                        vna = vjs[:, t].rearrange("c n d -> c (n d)")
                        vjf = vj[:].rearrange("c n d -> c (n d)")
                        uf = u_t[:].rearrange("c n d -> c (n d)")
                        nc.vector.scalar_tensor_tensor(
                            vna, vjf, dec, uf, op.mult, op.add)
                    if head:
                        r = r0 + i
                        nc.vector.tensor_tensor(
                            hvn[:, r], hv[:], hstg[:, r], op.add)
                    if jeff:
                        nc.vector.scalar_tensor_tensor(
                            vjf, vna, 1.0, vna, op.is_lt, op.mult)
                    if tail:
                        rt = tr0 + i
                        nc.vector.tensor_tensor(
                            tvn[:, rt], tv[:], tstg[:, rt], op.add)
                    if head:
                        nc.vector.scalar_tensor_tensor(
                            hv[:], hvn[:, r], 1.0, hvn[:, r],
                            op.is_lt, op.mult)
                    if tail:
                        nc.vector.scalar_tensor_tensor(
                            tv[:], tvn[:, rt], 1.0, tvn[:, rt],
                            op.is_lt, op.mult)

            nc.sync.dma_start(
                out_d[:].rearrange("n t -> (n t)").unsqueeze(0),
                outs[:].rearrange("p n t -> p (n t)"))

    nc.compile()
    return nc, out_d.name


def _prep_core_inputs(x, w1, b1, w2, b2, sw0, sb0, sw1, sb1, sw2, sb2,
                      core):
    """Host-side marshalling of one core's shard into device layouts."""
    n0 = core * NL
    inv = F32(1.0) / TAU
    xs = x[:, n0:n0 + NL]                      # (T, NL, 2, C)
    x0 = np.ascontiguousarray(np.moveaxis(xs[:, :, 0, :], [0, 1, 2],
                                          [2, 1, 0]))   # (C, NL, T)
    x1 = np.ascontiguousarray(np.moveaxis(xs[:, :, 1, :], [0, 1, 2],
                                          [2, 1, 0]))
    xa = np.zeros((C, NL, 128), F16)
    xa[:, :, :T] = (x0[:, :, ::-1] * inv).astype(F16)
    xb = np.zeros((C, NL, 128), F16)
    xb[:, :, 63:127] = (x1 * inv).astype(F16)

    dist = np.arange(D) - D // 2
    with np.errstate(divide="ignore"):
        kint = 1.0 / (1.0 - np.exp(-np.abs(dist) / 2.0))
    kint[D // 2] = 1.0 / (1.0 - np.exp(-1.0 / 2.0))
    kint16 = kint.astype(F16)
    kch = np.broadcast_to(kint16, (C, CH, NL, D)).reshape(C, -1).copy()

    # fp16 weight planes, laid out (t8, n, k)
    w1b = np.broadcast_to(w1[:, 0].astype(F16)[None, None, None, :],
                          (C, CH, NL, 10)).reshape(C, -1).copy()
    b1b = np.broadcast_to(b1.astype(F16)[None, None, None, :],
                          (C, CH, NL, 10)).reshape(C, -1).copy()
    w2b = np.broadcast_to(w2[0, :].astype(F16)[None, None, None, :],
                          (C, CH, NL, 10)).reshape(C, -1).copy()

    cpk = np.zeros((C, 4), F32)
    cpk[:, 0] = np.float16(w2[0, :]).astype(F32).sum()   # sum_k w2 (fp16)
    cpk[:, 1] = b2[0]

    spk = np.zeros((32, 40), F32)
    spk[:, 0:32] = sw1.T                       # stat for q1 matmul
    spk[:, 32] = sw0[:, 0] + 2.0 * sb0         # q0 comb
    spk[:, 33] = sw1.sum(axis=1)               # rowsum sw1 (g1 fix)
    spk[:, 34] = sb1
    spk[:, 35] = sw2[0, :]                     # stat for q2 matmul
    spk[:, 36] = (sw0[:, 0] + 2.0 * sb0) / 2.0
    spk[:, 37] = sw1.sum(axis=1) / 2.0

    srow = np.zeros((1, 40), F32)
    srow[0, 0:32] = sw0[:, 0]                  # stat for q0 matmul
    srow[0, 32] = sw2[0, :].sum()              # g2 fix
    srow[0, 33] = sb2[0]
    srow[0, 34] = sw2[0, :].sum() / 2.0

    onec = np.ones((C, 2), F16)

    return {
        "xa": xa, "xb": xb, "kch": kch, "w1b": w1b, "b1b": b1b,
        "w2b": w2b, "cpk": cpk, "spk": spk, "srow": srow, "onec": onec,
    }


def kernel(x, w1, b1, w2, b2, sw0, sb0, sw1, sb1, sw2, sb2):
    from concourse.bass_utils import run_bass_kernel_spmd

    if "prog" not in _cache:
        _cache["prog"] = _build_program()
    nc, out_name = _cache["prog"]

    args = (x.astype(F32), w1, b1, w2, b2, sw0, sb0, sw1, sb1, sw2, sb2)
    in_maps = [_prep_core_inputs(*args, core) for core in range(NCORES)]
    res = run_bass_kernel_spmd(nc, in_maps, core_ids=list(range(NCORES)))
    out = np.concatenate([r[out_name] for r in res.results], axis=0)
    # device layout is (N, T); reference returns (T, N, 1)
    return np.ascontiguousarray(out.T)[:, :, None].astype(F32)


if __name__ == "__main__":
    d = np.load("/tmp/inputs.npz")
    out = kernel(**{k: d[k] for k in d.files})
    print("kernel out", out.shape, float(np.abs(out).max()))
